# revision 56
# baseline (speedup 1.0000x reference)
"""Trainium2 Bass kernel for nn_Attention_2284922602161 (linear attention).

Math per batch element b (C=512, Cq=64, N=4096):
    Q = Wq@x + bq            [Cq, N]
    K = Wk@x + bk            [Cq, N]
    V = Wv@x + bv            [C, N]
    Qn = Q / ||Q||_col ; Kn = K / ||K||_col      (L2 over channel dim per position)
    ksum = sum_n Kn + eps    [Cq]
    tailor[n] = 1 / (N + Qn[:,n].ksum)
    M1 = Kn @ V^T            [Cq, C]
    out[:,n] = gamma * tailor[n] * (V.sum(-1) + M1^T @ Qn[:,n])

Sharding: pure data-parallel, one batch element per NeuronCore (B=8 over 8 cores).

v8 dataflow — V never materialized, x loaded ONCE (bf16):
    M1_aug^T = A_augT @ ... with A_augT[c,m] accumulated directly as four
    [128,65] PSUM accumulators: per n-tile, PE transposes the four xa
    sub-tiles (bf16, 53ns each) to get x^T on positions-partitions, then
    A_T[k] += xt[:, k-slice]^T_stationary @ [Kn^T|1].
    M1_aug = (A_augT-slices)^T @ WvT + [ksum;N] (x) bv.
    Q,K biases fold into the projection matmul as a rank-1 ones-row
    update, so PSUM evictions are plain copies and Act reads PSUM.
    Everything that depends only on Q/K (ksum row via ones^T @ kn_red,
    tailor scalars, qs scaling, output transposes + qf evictions) runs
    while the tail of x still streams; after the last A matmul only
    at->M1->m1_sb gates the out pass. Output bf16, one DMA per
    512-column group, PSUM evicted in [P,1024] pairs on Act/DVE.
"""

import numpy as np

B, C, H, W = 8, 512, 64, 64
N = H * W              # 4096
CQ = 64
P = 128
NT = N // P            # 32 n-tiles
KT = C // P            # 4 contraction tiles
QK = 2 * CQ            # 128 projected q|k channels
EPS = 1e-6
NCHUNK = 512
XCHUNKS = (4, 4, 8, 8, 8)
ALAG = 3               # A-matmul pipeline lag behind QK/transpose
_CACHE = {}


def _build():
    import concourse.bacc as bacc
    import concourse.mybir as mybir
    import concourse.tile as tile
    import concourse.bass as bass
    from contextlib import ExitStack

    f32 = mybir.dt.float32
    f32r = mybir.dt.float32r
    bf16 = mybir.dt.bfloat16

    nc = bacc.Bacc("TRN2", target_bir_lowering=False, debug=False,
                   enable_asserts=True, num_devices=8)

    xa_d = nc.declare_dram_parameter("xa", [P, NT, C], bf16, isOutput=False)
    xb_d = nc.declare_dram_parameter("xb", [P, NT, C], bf16, isOutput=False)
    wqk_d = nc.declare_dram_parameter("wqk", [C, QK], bf16, isOutput=False)
    # consts split by NATIVE dtype — fp32r matmul operands must be written
    # as f32r and bf16 matmul operands as bf16 (bitcasts are rejected)
    idb_d = nc.declare_dram_parameter("identb", [P, P + NT], bf16,
                                      isOutput=False)  # identb | ones32
    cprb_d = nc.declare_dram_parameter("cprb", [1, 2 * P], bf16,
                                       isOutput=False)  # brow | ones1p
    ckr_d = nc.declare_dram_parameter("ckr", [P, 327], f32r,
                                      isOutput=False)  # identf|onescol|knred0
    cprr_d = nc.declare_dram_parameter("cprr", [1, 640], f32r,
                                       isOutput=False)  # bvrow | onesrow
    wvt_d = nc.declare_dram_parameter("wvt", [P, KT * C], bf16,
                                      isOutput=False)
    out_d = nc.declare_dram_parameter("out", [C, N], bf16, isOutput=True)

    with tile.TileContext(nc) as tc:
        with ExitStack() as ctx:
            const = ctx.enter_context(tc.tile_pool(name="const", bufs=1))
            xapool = ctx.enter_context(tc.tile_pool(name="xapool", bufs=3))
            xbpool = ctx.enter_context(tc.tile_pool(name="xbpool", bufs=2))
            scpool = ctx.enter_context(tc.tile_pool(name="scpool", bufs=3))
            obufp = ctx.enter_context(tc.tile_pool(name="obufp", bufs=3))

            xa_ap = xa_d.ap()
            wqk_sb = const.tile([P, KT, QK], bf16)
            nc.sync.dma_start(
                out=wqk_sb, in_=wqk_d.ap().rearrange("(k p) w -> p k w", p=P))
            # consts on the Pool queue (SWDGE): small/early ones first
            idb_sb = const.tile([P, P + NT], bf16)
            nc.gpsimd.dma_start(out=idb_sb, in_=idb_d.ap())
            identb_sb = idb_sb[:, 0:P]
            ones32_sb = idb_sb[:, P:P + NT]
            cprb_sb = const.tile([1, 2 * P], bf16)
            nc.gpsimd.dma_start(out=cprb_sb, in_=cprb_d.ap())
            brow_sb = cprb_sb[0:1, 0:P]
            ones1p_sb = cprb_sb[0:1, P:2 * P]
            ckr_sb = const.tile([P, 327], f32r)
            nc.gpsimd.dma_start(out=ckr_sb, in_=ckr_d.ap())
            identf_sb = ckr_sb[:, 0:128]
            onescol_sb = ckr_sb[:, 128:129]
            cprr_sb = const.tile([1, 640], f32r)
            nc.gpsimd.dma_start(out=cprr_sb, in_=cprr_d.ap())
            bvrow_sb = cprr_sb[0:1, 0:512]
            onesrow_sb = cprr_sb[0:1, 512:640]
            wvt_sb = const.tile([P, KT, C], bf16)
            nc.gpsimd.dma_start(
                out=wvt_sb,
                in_=wvt_d.ap().rearrange("p (k c) -> p k c", k=KT))

            # --- persistent per-batch buffers ---
            q_all = const.tile([P, NT, CQ], bf16)       # biased Q^T
            kn_all = const.tile([P, NT, CQ + 1], bf16)  # [Kn^T | 1]
            # 66-wide slices (fp32r matmul moving size must be even);
            # carved from the DMA-zeroed f32r const so pad columns are 0
            kn_red = ckr_sb[:, 129:327].rearrange("p (h m) -> p h m", h=3)
            qs_all = const.tile([P, NT, CQ + 1], f32r)  # [s*Q^T | gamma*tailor]
            q_ss = const.tile([P, NT], f32)
            qd = const.tile([P, NT], f32)
            qsq = const.tile([P, NT, CQ], bf16)         # scratch squares/prods
            qf_all = const.tile([CQ + 1, N], f32r)      # transposed qs
            a_sb = const.tile([CQ + 1, C], bf16)        # A_aug evicted
            at_sb = const.tile([P, KT, CQ + 1], bf16)   # A_aug^T slices
            m1_sb = const.tile([CQ + 1, C], f32r)
            ks_eps = const.tile([P, CQ], f32)
            ksrow = const.tile([1, CQ + 1], f32r)
            M = CQ + 1

            # aug ones column for all kn tiles in one strided write
            nc.vector.tensor_copy(out=kn_all[:, :, CQ], in_=ones32_sb)
            # preload activation tables (Square/Sqrt) while DMAs stream
            warm = const.tile([1, 1], f32)
            nc.scalar.activation(out=warm, in_=wqk_sb[0:1, 0, 0:1],
                                 func=mybir.ActivationFunctionType.Square)
            nc.scalar.activation(out=warm, in_=warm,
                                 func=mybir.ActivationFunctionType.Sqrt)

            xb_ap = xb_d.ap()
            with tc.tile_pool(name="psQK", bufs=3, space="PSUM") as psQKp, \
                 tc.tile_pool(name="psA", bufs=1, space="PSUM") as psAp, \
                 tc.tile_pool(name="psT", bufs=2, space="PSUM") as psT_pool, \
                 tc.tile_pool(name="psB", bufs=1, space="PSUM") as psB:

                # ---- loop 1: stream XA; QK projection + kn chains --------
                xa_g = None
                x_base = 0
                xc = 0
                for i in range(NT):
                    if sum(XCHUNKS[:xc]) == i:
                        g = XCHUNKS[xc]
                        xa_g = xapool.tile([P, g, KT, P], bf16,
                                           name=f"xa_{xc}", tag="xa")
                        nc.sync.dma_start(
                            out=xa_g,
                            in_=xa_ap[:, i:i + g, :].rearrange(
                                "p g (k n) -> p g k n", k=KT))
                        x_base = i
                        xc += 1
                    xa_t = xa_g[:, i - x_base]

                    psqk = psQKp.tile([P, QK], f32, name=f"qk_{i}", tag="qk")
                    for k in range(KT):
                        nc.tensor.matmul(psqk, xa_t[:, k, :], wqk_sb[:, k, :],
                                         start=(k == 0), stop=False)
                    # fold Q,K biases in as rank-1 ones^T (x) [bq|bk]
                    nc.tensor.matmul(psqk, ones1p_sb, brow_sb,
                                     start=False, stop=True)
                    # Q evict first on DVE (independent of the K chain)
                    nc.vector.tensor_copy(out=q_all[:, i, :],
                                          in_=psqk[:, 0:CQ])
                    # K chain: kss -> 1/||K|| -> kn (bf16), straight off PSUM
                    sck = scpool.tile([P, CQ], f32, name=f"sck_{i}", tag="sck")
                    kss = scpool.tile([P, 1], f32, name=f"kss_{i}", tag="kss")
                    nc.scalar.activation(
                        out=sck, in_=psqk[:, CQ:QK],
                        func=mybir.ActivationFunctionType.Square,
                        accum_out=kss)
                    krt = scpool.tile([P, 1], f32, name=f"krt_{i}", tag="krt")
                    nc.scalar.activation(out=krt, in_=kss,
                                         func=mybir.ActivationFunctionType.Sqrt)
                    krs = scpool.tile([P, 1], f32, name=f"krs_{i}", tag="krs")
                    nc.vector.reciprocal(out=krs, in_=krt)
                    # GPSIMD cannot read PSUM: kn scale on DVE, Q copy on Act
                    nc.vector.tensor_scalar_mul(out=kn_all[:, i, 0:CQ],
                                                in0=psqk[:, CQ:QK],
                                                scalar1=krs)
                    # batched q squares + kn partial sums: 0..15 at i=15,
                    # 16..30 at i=30, tile 31 separately (so the mid-phase
                    # barrier chain only waits on tile 31's own tiny ops)
                    if i in (NT // 2 - 1, NT - 2):
                        h = 0 if i < NT // 2 else 1
                        h0 = h * (NT // 2)
                        h1 = (h + 1) * (NT // 2) - h
                        with nc.allow_low_precision(reason="q squares bf16"):
                            nc.vector.tensor_mul(out=qsq[:, h0:h1, :],
                                                 in0=q_all[:, h0:h1, :],
                                                 in1=q_all[:, h0:h1, :])
                        qsh = q_ss[:, h0:h1]
                        qss3 = bass.AP(tensor=qsh.tensor, offset=qsh.offset,
                                       ap=[qsh.ap[0], qsh.ap[1], [1, 1]])
                        nc.vector.reduce_sum(out=qss3, in_=qsq[:, h0:h1, :],
                                             axis=mybir.AxisListType.X)
                        ksl = kn_all[:, h0:h1, :]
                        ksw = bass.AP(tensor=ksl.tensor, offset=ksl.offset,
                                      ap=[ksl.ap[0], ksl.ap[2], ksl.ap[1]])
                        krl = kn_red[:, h, 0:CQ + 1]
                        kr3 = bass.AP(tensor=krl.tensor, offset=krl.offset,
                                      ap=[krl.ap[0], krl.ap[1], [1, 1]])
                        with nc.allow_low_precision(reason="f32r ksum"):
                            nc.vector.reduce_sum(out=kr3, in_=ksw,
                                                 axis=mybir.AxisListType.X)
                    if i == NT - 1:
                        nc.vector.tensor_copy(out=kn_red[:, 2, 0:CQ + 1],
                                              in_=kn_all[:, i, :])
                        scq = scpool.tile([P, CQ], bf16, name="scq31",
                                          tag="scq")
                        with nc.allow_low_precision(reason="scratch sq"):
                            nc.scalar.activation(
                                out=scq, in_=q_all[:, i, :],
                                func=mybir.ActivationFunctionType.Square,
                                accum_out=q_ss[:, i:i + 1])

                # ---- loop 3: stream XB; A accumulation; tr/qf groups
                # interleaved into the PE queue so neither blocks the other
                a_ps = psAp.tile([M, C], f32)
                xb_g = None
                xc = 0
                for j in range(NT):
                    if sum(XCHUNKS[:xc]) == j:
                        g = XCHUNKS[xc]
                        xb_g = xbpool.tile([P, g, C], bf16,
                                           name=f"xb_{xc}", tag="xb")
                        nc.sync.dma_start(out=xb_g, in_=xb_ap[:, j:j + g, :])
                        x_base = j
                        xc += 1
                    nc.tensor.matmul(a_ps, kn_all[:, j, :],
                                     xb_g[:, j - x_base],
                                     start=(j == 0), stop=(j == NT - 1))
                # ---- ksum row + tailor scalars (run during XB) -------
                # [ksum; N] row: ones^T @ kn_red (3 slices), add slices
                MP = CQ + 2
                ksr_ps = psB.tile([1, 3 * MP], f32)
                nc.tensor.matmul(
                    ksr_ps, onescol_sb,
                    kn_red.rearrange("p h m -> p (h m)"),
                    start=True, stop=True)
                ksr_sb = const.tile([1, 3 * MP], f32r)
                nc.vector.tensor_copy(out=ksr_sb, in_=ksr_ps)
                ksrow2 = const.tile([1, M], f32r)
                nc.vector.tensor_add(out=ksrow2, in0=ksr_sb[0:1, 0:M],
                                     in1=ksr_sb[0:1, MP:MP + M])
                nc.vector.tensor_add(out=ksrow, in0=ksrow2,
                                     in1=ksr_sb[0:1, 2 * MP:2 * MP + M])
                rep_ps = psB.tile([P, CQ], f32)
                nc.tensor.matmul(rep_ps, onesrow_sb, ksrow[0:1, 0:CQ],
                                 start=True, stop=True)
                nc.vector.tensor_scalar_add(out=ks_eps, in0=rep_ps,
                                            scalar1=EPS)
                ksb = const.tile([P, CQ], bf16)
                nc.vector.tensor_copy(out=ksb, in_=ks_eps)

                # tailor scalars
                qrt_all = const.tile([P, NT], f32)
                nc.scalar.activation(out=qrt_all, in_=q_ss,
                                     func=mybir.ActivationFunctionType.Sqrt)
                qrs_all = const.tile([P, NT], f32)
                nc.vector.reciprocal(out=qrs_all, in_=qrt_all)
                ksb_ap = bass.AP(tensor=ksb.tensor, offset=ksb.offset,
                                 ap=[ksb.ap[0], [0, NT], [1, CQ]])
                nc.vector.tensor_mul(out=qsq, in0=q_all, in1=ksb_ap)
                nc.vector.reduce_sum(out=qd, in_=qsq,
                                     axis=mybir.AxisListType.X)
                dn = const.tile([P, NT], f32)
                nc.vector.tensor_mul(out=dn, in0=qd, in1=qrs_all)
                nc.vector.tensor_scalar_add(out=dn, in0=dn, scalar1=float(N))
                tailor = const.tile([P, NT], f32)
                nc.vector.reciprocal(out=tailor, in_=dn)
                s_all = const.tile([P, NT], bf16)
                nc.vector.tensor_mul(out=s_all, in0=tailor, in1=qrs_all)
                nc.vector.tensor_copy(out=qs_all[:, :, CQ], in_=tailor)

                NG = NCHUNK // P  # 4 tiles per group
                NGR = NT // NG
                # qs scale muls, alternating DVE/Pool
                for g in range(NGR):
                    s_sl = s_all[:, g * NG:(g + 1) * NG]
                    eng = nc.vector if g % 2 == 0 else nc.gpsimd
                    eng.tensor_mul(
                        out=qs_all[:, g * NG:(g + 1) * NG, 0:CQ],
                        in0=q_all[:, g * NG:(g + 1) * NG, :],
                        in1=bass.AP(tensor=s_sl.tensor, offset=s_sl.offset,
                                    ap=[s_sl.ap[0], s_sl.ap[1], [0, CQ]]))
                def tr_qf(g):
                    tr_ps = psT_pool.tile([CQ + 1, NG, P], f32r,
                                          name=f"tr_{g}", tag="tr")
                    for u in range(NG):
                        nc.tensor.transpose(tr_ps[:, u, :],
                                            qs_all[:, g * NG + u, :],
                                            identf_sb)
                    dstq = qf_all[:, g * NCHUNK:(g + 1) * NCHUNK]
                    srcq = tr_ps.rearrange("m u n -> m (u n)")
                    if g % 2 == 0:
                        nc.scalar.copy(out=dstq, in_=srcq)
                    else:
                        nc.vector.tensor_copy(out=dstq, in_=srcq)

                for g in range(NGR):
                    tr_qf(g)
                # A_aug -> bf16 sbuf (Act)
                nc.scalar.copy(out=a_sb, in_=a_ps)

            # ---- A^T -> M1 (needs its own PSUM banks) --------------------
            with tc.tile_pool(name="psB2", bufs=1, space="PSUM") as psB2:
                # 66-wide slices keep bf16 PSUM accesses 4B aligned
                at_ps2 = psB2.tile([P, KT, M + 1], bf16)
                for k in range(KT):
                    nc.tensor.transpose(
                        at_ps2[:, k, 0:M], a_sb[:, k * P:(k + 1) * P],
                        identb_sb[0:M, 0:M])
                nc.vector.tensor_copy(out=at_sb, in_=at_ps2[:, :, 0:M])
                m1_ps = psB2.tile([M, C], f32)
                for k in range(KT):
                    nc.tensor.matmul(m1_ps, at_sb[:, k, :], wvt_sb[:, k, :],
                                     start=(k == 0), stop=False)
                nc.tensor.matmul(m1_ps, ksrow, bvrow_sb,
                                 start=False, stop=True)
                nc.scalar.copy(out=m1_sb, in_=m1_ps)

            # ---- out pass: matmuls, pair evictions, one DMA/group --------
            with tc.tile_pool(name="psE", bufs=4, space="PSUM") as psE_pool:
                for g in range(NT // NG):
                    dst = obufp.tile([P, KT, NCHUNK], bf16,
                                     name=f"ob_{g}", tag="ob")
                    for half in range(2):
                        out_ps = psE_pool.tile([P, 2, NCHUNK], f32,
                                               name=f"ops_{half}_{g}",
                                               tag="ops")
                        for ci in range(2):
                            c = half * 2 + ci
                            nc.tensor.matmul(
                                out_ps[:, ci, :],
                                m1_sb[:, c * P:(c + 1) * P],
                                qf_all[:, g * NCHUNK:(g + 1) * NCHUNK],
                                start=True, stop=True)
                        dsth = dst[:, half * 2:half * 2 + 2, :]
                        if half == 0:
                            nc.scalar.copy(out=dsth, in_=out_ps)
                        else:
                            nc.vector.tensor_copy(out=dsth, in_=out_ps)
                    nc.sync.dma_start(
                        out=out_d.ap()[:, g * NCHUNK:(g + 1) * NCHUNK]
                        .rearrange("(c p) n -> p c n", p=P),
                        in_=dst)

    nc.compile()
    return nc


def _get_nc():
    if "nc" not in _CACHE:
        _CACHE["nc"] = _build()
    return _CACHE["nc"]


def _prep_inputs(x, Wq, bq, Wk, bk, Wv, bv, gamma):
    import ml_dtypes
    bf = ml_dtypes.bfloat16
    x = np.ascontiguousarray(np.asarray(x, dtype=np.float32)).reshape(B, C, N)
    # XA[b, p, i, k*128+j] = x[b, k*128+p, i*128+j]
    xa = np.ascontiguousarray(
        x.reshape(B, KT, P, NT, P).transpose(0, 2, 3, 1, 4)
        .reshape(B, P, NT, C).astype(bf))
    # XB[b, p, i, c] = x[b, c, i*128+p]
    xb = np.ascontiguousarray(
        x.reshape(B, C, NT, P).transpose(0, 3, 2, 1).astype(bf))
    wqk = np.ascontiguousarray(np.concatenate(
        [np.asarray(Wq, np.float32).T,
         np.asarray(Wk, np.float32).T], axis=1).astype(bf))
    # wvt[p, k, c] = gamma * Wv.T[k*128+p, c]  (gamma folded in)
    gscal = np.float32(np.asarray(gamma).reshape(-1)[0])
    wvt = np.ascontiguousarray(
        (np.asarray(Wv, np.float32).T * gscal).reshape(KT, P, C)
        .transpose(1, 0, 2).reshape(P, KT * C).astype(bf))
    ckr = np.concatenate([
        np.eye(P, dtype=np.float32),                             # identf
        np.ones((P, 1), dtype=np.float32),                       # onescol
        np.zeros((P, 198), dtype=np.float32),                    # kn_red init
    ], axis=1)
    cprb = np.concatenate([
        np.concatenate([np.asarray(bq, np.float32),
                        np.asarray(bk, np.float32)])[None, :],   # brow
        np.ones((1, P), dtype=np.float32),                       # ones1p
    ], axis=1).astype(bf)
    cprr = np.concatenate([
        (np.asarray(bv, np.float32) * gscal)[None, :],           # gamma*bv
        np.ones((1, P), dtype=np.float32),                       # onesrow
    ], axis=1)
    return {
        "xa": xa,
        "xb": xb,
        "wqk": wqk,
        "identb": np.concatenate(
            [np.eye(P, dtype=np.float32),
             np.ones((P, NT), dtype=np.float32)], axis=1).astype(bf),
        "cprb": np.ascontiguousarray(cprb),
        "ckr": np.ascontiguousarray(ckr),
        "cprr": np.ascontiguousarray(cprr),
        "wvt": wvt,
    }


def kernel(x, Wq, bq, Wk, bk, Wv, bv, gamma, _trace=False):
    from concourse.bass_utils import run_bass_kernel_spmd

    common = _prep_inputs(x, Wq, bq, Wk, bk, Wv, bv, gamma)
    xa = common.pop("xa")
    xb = common.pop("xb")
    nc = _get_nc()
    in_maps = [{"xa": xa[i], "xb": xb[i], **common} for i in range(B)]
    res = run_bass_kernel_spmd(nc, in_maps, list(range(B)), trace=_trace)
    out = np.stack([np.asarray(res.results[i]["out"]).astype(np.float32)
                    for i in range(B)])
    if _trace:
        _CACHE["last_results"] = res
    return out.reshape(B, C, H, W)
